# revision 1
# baseline (speedup 1.0000x reference)
"""MoE grouped-GEMM kernel for Trainium2 (8 NeuronCores, expert-parallel).

Problem: x [16384, 1024] fp16, expert_indices [16384] int32 (0..7),
weights [8, 1024, 4096] fp16. Output: fp16 [16384, 4096] in sorted-token
order (stable sort by expert), fp32 accumulation.

Sharding: the host performs the argsort/bincount dispatch (that IS the
sharding step) and gives core e the tokens routed to expert e as a
pre-transposed xT [K, Mpad] fp16 block plus that expert's weights
[K, N]. Every core runs the identical dense-GEMM program (token counts
padded to a common multiple of 128), so a single SPMD NEFF drives all 8
cores with no device-side collectives. The host concatenates the
per-expert output blocks, which is exactly sorted-token order.
"""

import numpy as np

_NCORES = 8


def _build_program(T, K, N):
    """Dense GEMM per core: out[Mpad, N] = xT.T @ w, fp32 PSUM accumulation.

    Layout per core:
      xT [K, Mpad] fp16  (x pre-transposed on host so K lands on partitions)
      w  [K, N]   fp16
      out [Mpad, N] fp16, Mpad = T*128

    PE mapping: stationary lhsT = xT k-tile [128, 128], moving rhs = w
    [128, 512] slice, PSUM [128m, 512n] fp32 accumulated over K/128
    k-tiles. PSUM is split into two 4-bank halves (bufs=2) so the
    DVE fp32->fp16 eviction of one half overlaps matmuls in the other.
    """
    from concourse import bacc, bass, tile
    import concourse.mybir as mybir

    f16 = mybir.dt.float16
    f32 = mybir.dt.float32
    Mpad = T * 128
    KT = K // 128            # k-tiles (contraction)
    NB = 512                 # one PSUM bank of fp32
    NH = 2048                # psum half (4 banks)
    nhalves = N // NH

    nc = bacc.Bacc(
        "TRN2", target_bir_lowering=False, debug=False, num_devices=_NCORES
    )
    xT = nc.dram_tensor("xT", [K, Mpad], f16, kind="ExternalInput").ap()
    w = nc.dram_tensor("w", [K, N], f16, kind="ExternalInput").ap()
    out = nc.dram_tensor("out", [Mpad, N], f16, kind="ExternalOutput").ap()

    with tile.TileContext(nc) as tc:
        with (
            tc.tile_pool(name="xw", bufs=1) as xw,
            tc.tile_pool(name="op", bufs=3) as op,
            tc.tile_pool(name="pp", bufs=2, space=bass.MemorySpace.PSUM) as pp,
        ):
            # Whole x and w stay SBUF-resident (~99KB/partition total).
            # Interleave x strips with first-half w strips so the first
            # (t=0, h=0) accumulation can start as soon as strip k=0 lands.
            xs = []
            ws = [[None] * nhalves for _ in range(KT)]
            for k in range(KT):
                xt = xw.tile([128, Mpad], f16, tag=f"x{k}")
                nc.sync.dma_start(xt[:], xT[k * 128 : (k + 1) * 128, :])
                xs.append(xt)
                wt = xw.tile([128, NH], f16, tag=f"w{k}h0")
                nc.sync.dma_start(wt[:], w[k * 128 : (k + 1) * 128, 0:NH])
                ws[k][0] = wt
            for h in range(1, nhalves):
                for k in range(KT):
                    wt = xw.tile([128, NH], f16, tag=f"w{k}h{h}")
                    nc.sync.dma_start(
                        wt[:], w[k * 128 : (k + 1) * 128, h * NH : (h + 1) * NH]
                    )
                    ws[k][h] = wt

            for t in range(T):
                ot = op.tile([128, N], f16, tag="ot")
                for h in range(nhalves):
                    ps = pp.tile([128, NH], f32, tag="ps")
                    for k in range(KT):
                        lhs = xs[k][:, t * 128 : (t + 1) * 128]
                        for n in range(NH // NB):
                            nc.tensor.matmul(
                                ps[:, n * NB : (n + 1) * NB],
                                lhs,
                                ws[k][h][:, n * NB : (n + 1) * NB],
                                start=(k == 0),
                                stop=(k == KT - 1),
                            )
                    nc.vector.tensor_copy(ot[:, h * NH : (h + 1) * NH], ps[:])
                nc.sync.dma_start(out[t * 128 : (t + 1) * 128, :], ot[:])
    nc.compile()
    return nc


# test.py reads these after a call for timing/trace introspection
last_results = None


def kernel(x, expert_indices, weights):
    x = np.asarray(x)
    ei = np.asarray(expert_indices)
    w = np.asarray(weights)
    M, K = x.shape
    E, K2, N = w.shape
    assert K == K2 and E == _NCORES

    counts = np.bincount(ei, minlength=E)
    T = max(1, -(-int(counts.max()) // 128))
    Mpad = T * 128
    order = np.argsort(ei, kind="stable")
    x_sorted = x[order]
    offs = np.zeros(E + 1, dtype=np.int64)
    np.cumsum(counts, out=offs[1:])

    in_maps = []
    for e in range(E):
        blk = x_sorted[offs[e] : offs[e + 1]]
        xeT = np.zeros((K, Mpad), dtype=np.float16)
        xeT[:, : blk.shape[0]] = blk.T
        in_maps.append({"xT": xeT, "w": np.ascontiguousarray(w[e])})

    nc = _build_program(T, K, N)

    from concourse.bass_utils import run_bass_kernel_spmd

    res = run_bass_kernel_spmd(nc, in_maps, list(range(E)))
    global last_results
    last_results = res

    out = np.empty((M, N), dtype=np.float16)
    for e in range(E):
        out[offs[e] : offs[e + 1]] = res.results[e]["out"][: counts[e]]
    return out
